# revision 4
# baseline (speedup 1.0000x reference)
"""DiagonalSSM Trainium2 kernel, v2: software-pipelined emission order.

Same math as v1 (see kernel.py docstring). Changes:
  - x quarter 0 is loaded/transposed before weight prep so PE starts sooner
  - transposes of quarter q+1 are emitted before matmuls of quarter q
    (per-engine streams are in-order; this removes PE stalls at quarter
    boundaries waiting on DVE scans)
  - XPOSE_MODE selects the PE transpose dtype path:
      "f32"  - plain fp32 transpose (2 cyc/row, exact)
      "f32r" - float32r-tagged transpose (1.5 cyc/row, bit-exact if HW agrees)
      "bf16" - pre-cast x to bf16 on ACT, 1 cyc/row, small accuracy loss
      "fp16" - pre-cast x to fp16 on GpSimd (idle engine), 1 cyc/row,
               8x less x-rounding error than bf16
"""

import contextlib

import numpy as np

import concourse.bacc as bacc
import concourse.mybir as mybir
from concourse import masks
from concourse.tile import TileContext
from concourse.bass_utils import run_bass_kernel_spmd

B, T, D = 8, 4096, 1024
P = 128
NDC = D // P
Q = 512
NQ = T // Q
NTC = Q // P
GW = 512              # t per transpose group (4 t-chunks -> one psum bank)
NG = Q // GW
EH = 512
F32 = mybir.dt.float32
F32R = mybir.dt.float32r
BF16 = mybir.dt.bfloat16
FP16 = mybir.dt.float16
AF = mybir.ActivationFunctionType
ALU = mybir.AluOpType

XPOSE_MODE = "fp16"


def build_kernel(loop_n=None):
    nc = bacc.Bacc("TRN2", target_bir_lowering=False, debug=False, num_devices=B)
    x = nc.declare_dram_parameter("x", [T, D], F32, isOutput=False)
    a = nc.declare_dram_parameter("a", [D], F32, isOutput=False)
    b = nc.declare_dram_parameter("b", [D], F32, isOutput=False)
    w = nc.declare_dram_parameter("w_out", [D, D], F32, isOutput=False)
    bo = nc.declare_dram_parameter("b_out", [D], F32, isOutput=False)
    out = nc.declare_dram_parameter("out", [T, D], F32, isOutput=True)

    with TileContext(nc) as tc:
        with (
            tc.tile_pool(name="const", bufs=1) as cpool,
            tc.tile_pool(name="stage", bufs=10) as stage_pool,
            tc.tile_pool(name="xT", bufs=2) as xT_pool,
            tc.tile_pool(name="y", bufs=2) as y_pool,
            tc.tile_pool(name="wT", bufs=1) as wT_pool,
            tc.tile_pool(name="outs", bufs=3) as out_pool,
            tc.tile_pool(name="psA", bufs=4, space="PSUM") as psA,
            tc.tile_pool(name="psB", bufs=4, space="PSUM") as psB,
        ):
          loop_cm = (tc.For_i(0, loop_n, 1, hint_engines=(mybir.EngineType.PE,))
                     if loop_n else contextlib.nullcontext())
          with loop_cm:
            # ---------- constants ----------
            ident = cpool.tile([P, P], F32, tag="ident")
            masks.make_identity(nc, ident[:])

            a_tile = cpool.tile([P, NDC], F32, tag="a_t")
            nc.sync.dma_start(out=a_tile[:], in_=a[:].rearrange("(c p) -> p c", p=P))
            ah_tile = cpool.tile([P, NDC], F32, tag="ah_t")
            nc.scalar.activation(ah_tile[:], a_tile[:], AF.Tanh)
            b_tile = cpool.tile([P, NDC], F32, tag="b_t")
            nc.sync.dma_start(out=b_tile[:], in_=b[:].rearrange("(c p) -> p c", p=P))

            bo_row = cpool.tile([1, D], F32, tag="bo_row")
            nc.sync.dma_start(out=bo_row[:], in_=bo[:].rearrange("(o d) -> o d", o=1))
            bias_bc = cpool.tile([P, D], F32, tag="bias_bc")
            nc.gpsimd.partition_broadcast(bias_bc[:], bo_row[:])

            def pe_transpose(ps_slice, in_slice):
                if XPOSE_MODE == "f32r":
                    nc.tensor.transpose(
                        ps_slice.bitcast(F32R),
                        in_slice.bitcast(F32R),
                        ident[:].bitcast(F32R),
                    )
                else:
                    nc.tensor.transpose(ps_slice, in_slice, ident[:])

            def load_and_transpose_quarter(q, group_plan=None):
                """DMA x rows and PE-transpose into [d, t] tiles for quarter q.

                group_plan: list of chunk counts per transpose group (each
                chunk is 128 t rows); sum must be Q//P. Narrow first groups
                let the PE start before the whole 2MB group has landed.
                """
                xd = {"bf16": BF16, "fp16": FP16}.get(XPOSE_MODE, F32)
                xTq = [
                    xT_pool.tile([P, Q], xd, name=f"xT{dc}_{q}", tag=f"xT{dc}")
                    for dc in range(NDC)
                ]
                if group_plan is None:
                    group_plan = [GW // P] * NG
                assert sum(group_plan) == Q // P
                toff = 0
                for nk in group_plan:
                    xstages = []
                    for k in range(nk):
                        t0 = q * Q + (toff + k) * P
                        xs = stage_pool.tile([P, D], F32, tag="stage")
                        nc.sync.dma_start(out=xs[:], in_=x[t0 : t0 + P, :])
                        if XPOSE_MODE == "bf16":
                            xb = stage_pool.tile([P, D], BF16, tag="stage_bf")
                            nc.scalar.copy(xb[:], xs[:])
                            xs = xb
                        elif XPOSE_MODE == "fp16":
                            xb = stage_pool.tile([P, D], FP16, tag="stage_bf")
                            nc.scalar.copy(xb[:], xs[:])
                            xs = xb
                        xstages.append(xs)
                    for dc in range(NDC):
                        ps = psA.tile([P, nk * P], xd, tag="psA")
                        for k in range(nk):
                            if XPOSE_MODE in ("bf16", "fp16"):
                                nc.tensor.transpose(
                                    ps[:, k * P : (k + 1) * P],
                                    xstages[k][:, dc * P : (dc + 1) * P],
                                    ident_bf[:],
                                )
                            else:
                                pe_transpose(
                                    ps[:, k * P : (k + 1) * P],
                                    xstages[k][:, dc * P : (dc + 1) * P],
                                )
                        nc.scalar.copy(
                            xTq[dc][:, toff * P : (toff + nk) * P], ps[:]
                        )
                    toff += nk
                return xTq

            if XPOSE_MODE in ("bf16", "fp16"):
                xd16 = BF16 if XPOSE_MODE == "bf16" else FP16
                ident_bf = cpool.tile([P, P], xd16, tag="ident_bf")
                nc.vector.tensor_copy(ident_bf[:], ident[:])

            # ---------- x quarter 0 first: get PE going ASAP ----------
            xT_cur = load_and_transpose_quarter(0)

            # ---------- weight prep ----------
            wT = [
                wT_pool.tile([P, D], BF16, name=f"wT{dc}", tag=f"wT{dc}")
                for dc in range(NDC)
            ]
            for half in range(2):
                wstages = []
                for k in range(4):
                    ec = half * 4 + k
                    ws = stage_pool.tile([P, D], F32, tag="stage")
                    nc.sync.dma_start(out=ws[:], in_=w[ec * P : (ec + 1) * P, :])
                    wstages.append(ws)
                for dc in range(NDC):
                    ps = psA.tile([P, EH], F32, tag="psA")
                    for k in range(4):
                        pe_transpose(
                            ps[:, k * P : (k + 1) * P],
                            wstages[k][:, dc * P : (dc + 1) * P],
                        )
                    nc.scalar.activation(
                        wT[dc][:, half * EH : (half + 1) * EH],
                        ps[:],
                        AF.Copy,
                        scale=b_tile[:, dc : dc + 1],
                    )

            # ---------- main pipelined loop ----------
            y_prev = None
            for q in range(NQ):
                # scans for quarter q
                yq = [
                    y_pool.tile([P, Q], BF16, name=f"y{dc}_{q}", tag=f"y{dc}")
                    for dc in range(NDC)
                ]
                for dc in range(NDC):
                    data0 = ah_tile[:, dc : dc + 1].broadcast_to([P, Q])
                    initial = 0.0 if q == 0 else y_prev[dc][:, Q - 1 : Q]
                    nc.vector.tensor_tensor_scan(
                        out=yq[dc][:],
                        data0=data0,
                        data1=xT_cur[dc][:],
                        initial=initial,
                        op0=ALU.mult,
                        op1=ALU.add,
                    )

                # next quarter's transposes BEFORE this quarter's matmuls
                if q + 1 < NQ:
                    xT_cur = load_and_transpose_quarter(q + 1)

                # matmuls + bias + store for quarter q
                for t_c in range(NTC):
                    ostage = out_pool.tile([P, D], F32, tag="ostage")
                    for eh in range(2):
                        ps = psB.tile([P, EH], F32, tag="psB")
                        for dc in range(NDC):
                            nc.tensor.matmul(
                                ps[:],
                                lhsT=yq[dc][:, t_c * P : (t_c + 1) * P],
                                rhs=wT[dc][:, eh * EH : (eh + 1) * EH],
                                start=(dc == 0),
                                stop=(dc == NDC - 1),
                            )
                        nc.vector.tensor_add(
                            ostage[:, eh * EH : (eh + 1) * EH],
                            ps[:],
                            bias_bc[:, eh * EH : (eh + 1) * EH],
                        )
                    t0 = q * Q + t_c * P
                    nc.sync.dma_start(out=out[t0 : t0 + P, :], in_=ostage[:])

                y_prev = yq

    nc.finalize()
    return nc


_NC = None


def _get_nc():
    global _NC
    if _NC is None:
        _NC = build_kernel()
    return _NC


def kernel(x, a, b, w_out, b_out):
    x = np.ascontiguousarray(x, dtype=np.float32)
    a = np.ascontiguousarray(a, dtype=np.float32)
    b = np.ascontiguousarray(b, dtype=np.float32)
    w_out = np.ascontiguousarray(w_out, dtype=np.float32)
    b_out = np.ascontiguousarray(b_out, dtype=np.float32)
    nc = _get_nc()
    in_maps = [
        {"x": x[c], "a": a, "b": b, "w_out": w_out, "b_out": b_out} for c in range(B)
    ]
    res = run_bass_kernel_spmd(nc, in_maps, list(range(B)))
    return np.stack([res.results[c]["out"] for c in range(B)], axis=0)
